# revision 20
# baseline (speedup 1.0000x reference)
"""Cross-modal attention Trainium2 kernel.

Reference computation (all 1x1 convs + folded eval-mode BN):
  q = BN(Wq @ rgb), k = BN(Wk @ edge), v = BN(Wv @ edge)
  attn = softmax(q^T k) per head; xx = relu(attn @ v); out = BN(Wp @ xx)

Shapes: B=2, C=256, H=W=64 (N=4096), heads=8, key_dim=16, d=32.

Sharding: 8 cores = (batch b in {0,1}) x (query-slice qs in {0..3}, 1024
queries each). Each core computes K/V projections for the full N of its
batch (cheap) and attention + output projection for its query slice; the
host concatenates slices. No collectives.

Per-core dataflow (scores kept transposed so softmax-sum and the AV
contraction both run on the m axis without transposing the big matrix):
  sT[m, q] = sum_kd kk[kd, m] qq[kd, q]     (PE, fp32r, 32x128 row-tiled:
                                             2 heads concurrently)
  e = exp(sT)                               (ScalarE PSUM->SBUF bf16; max-
                                             subtraction skipped: |s|<~45)
  av[q, (h: d|den)] += e[mtile]^T @ [v|1]   (PE bf16, exp as stationary)
  xx = relu(av) * recip(den)                (VectorE, per-partition scalar)
  out = Wp^T @ xx^T + bp                    (PE transpose + fp32r matmul)
"""

import sys

for p in ("/opt/trn_rl_repo", "/opt/trn_rl_repo/concourse"):
    if p not in sys.path:
        sys.path.insert(0, p)

import numpy as np

import concourse.bass as bass
import concourse.mybir as mybir
import concourse.tile as tile
from concourse.bass_utils import run_bass_kernel_spmd
from concourse.tile_rust import add_dep_helper

F32 = mybir.dt.float32
F32R = mybir.dt.float32r
BF16 = mybir.dt.bfloat16
AF = mybir.ActivationFunctionType

NUM_HEADS, KD, D = 8, 16, 32
B, C, H, W = 2, 256, 64, 64
N = H * W            # 4096 keys per batch
QCH = 1024           # queries per core
NMT = N // 128       # 32 m-tiles
HB = 33              # per-head AV block: 32 v-cols + 1 denominator col


def build_nc(trace_scopes=False):
    nc = bass.Bass()

    rgb_s = nc.dram_tensor("rgb_s", [C, QCH], F32, kind="ExternalInput")
    edge = nc.dram_tensor("edge", [C, N], F32, kind="ExternalInput")
    w_qA = nc.dram_tensor("w_qA", [C, 128], F32, kind="ExternalInput")
    w_qB = nc.dram_tensor("w_qB", [C, 128], F32, kind="ExternalInput")
    w_kA = nc.dram_tensor("w_kA", [C, 128], F32, kind="ExternalInput")
    w_kB = nc.dram_tensor("w_kB", [C, 128], F32, kind="ExternalInput")
    w_v = nc.dram_tensor("w_v", [C, 264], F32, kind="ExternalInput")
    w_p = nc.dram_tensor("w_p", [256, C], F32, kind="ExternalInput")
    b_qA = nc.dram_tensor("b_qA", [128, 1], F32, kind="ExternalInput")
    b_qB = nc.dram_tensor("b_qB", [128, 1], F32, kind="ExternalInput")
    b_kA = nc.dram_tensor("b_kA", [128, 1], F32, kind="ExternalInput")
    b_kB = nc.dram_tensor("b_kB", [128, 1], F32, kind="ExternalInput")
    b_v = nc.dram_tensor("b_v", [128, 264], F32, kind="ExternalInput")
    b_p = nc.dram_tensor("b_p", [C, 1], F32, kind="ExternalInput")
    ident = nc.dram_tensor("ident", [128, 128], F32, kind="ExternalInput")
    out = nc.dram_tensor("out", [C, QCH], F32, kind="ExternalOutput")

    with tile.TileContext(nc) as tc:
        with tc.tile_pool(name="const", bufs=1) as cp, \
             tc.tile_pool(name="data", bufs=1) as dp:
            wq = [cp.tile([128, 256], F32R, name=f"wq{x}", tag=f"wq{x}") for x in "AB"]
            wk = [cp.tile([128, 256], F32R, name=f"wk{x}", tag=f"wk{x}") for x in "AB"]
            wv = cp.tile([128, 528], F32R, name="wv", tag="wv")
            wp = cp.tile([128, 512], F32R, name="wp", tag="wp")
            bq = [cp.tile([128, 1], F32, name=f"bq{x}", tag=f"bq{x}") for x in "AB"]
            bk = [cp.tile([128, 1], F32, name=f"bk{x}", tag=f"bk{x}") for x in "AB"]
            bv = cp.tile([128, 264], F32, name="bv", tag="bv")
            bp = cp.tile([128, 2], F32, name="bp", tag="bp")
            idn = cp.tile([128, 128], F32, name="idn", tag="idn")
            zrow = cp.tile([1, 128], BF16, name="zrow", tag="zrow")

            nc.sync.dma_start(bp[:, 0:1], b_p[0:128, :])
            nc.sync.dma_start(bp[:, 1:2], b_p[128:256, :])
            nc.sync.dma_start(bq[0][:], b_qA[:])
            nc.sync.dma_start(bq[1][:], b_qB[:])
            nc.sync.dma_start(bk[0][:], b_kA[:])
            nc.sync.dma_start(bk[1][:], b_kB[:])
            nc.sync.dma_start(bv[:], b_v[:])
            nc.vector.memset(zrow[:], 0.0)

            rgb_sb = dp.tile([128, 2 * QCH], F32R, name="rgb_sb", tag="rgb")
            edge_sb = [dp.tile([128, N], F32R, name=f"edge{k}", tag=f"edge{k}") for k in range(2)]

            # PE-consumed tensors bounce through one DVE copy each: a fused-
            # LDWEIGHTS (fp32/f32r) matmul can carry only ONE sync wait in
            # walrus codegen, so its cross-engine deps must collapse to the
            # single DVE semaphore; the copy also rounds fp32 -> fp32r.
            # (landing tiles stay alive the whole kernel: releasing them
            # would make later pools inherit the released-zone's DMA deps
            # as >1-wait instructions, which walrus rejects)
            l_wq = [dp.tile([128, 256], F32, name=f"l_wq{x}", tag=f"l_wq{x}") for x in "AB"]
            l_wk = [dp.tile([128, 256], F32, name=f"l_wk{x}", tag=f"l_wk{x}") for x in "AB"]
            l_wv = dp.tile([128, 528], F32, name="l_wv", tag="l_wv")
            l_wp = dp.tile([128, 512], F32, name="l_wp", tag="l_wp")
            l_id = dp.tile([128, 128], F32, name="l_id", tag="l_id")
            l_rgb = dp.tile([128, 2 * QCH], F32, name="l_rgb", tag="l_rgb")
            l_edge = [dp.tile([128, N], F32, name=f"l_edge{k}", tag=f"l_edge{k}") for k in range(2)]
            for k in range(2):
                nc.sync.dma_start(l_wq[0][:, 128 * k:128 * (k + 1)], w_qA[128 * k:128 * (k + 1), :])
                nc.sync.dma_start(l_wq[1][:, 128 * k:128 * (k + 1)], w_qB[128 * k:128 * (k + 1), :])
                nc.sync.dma_start(l_wk[0][:, 128 * k:128 * (k + 1)], w_kA[128 * k:128 * (k + 1), :])
                nc.sync.dma_start(l_wk[1][:, 128 * k:128 * (k + 1)], w_kB[128 * k:128 * (k + 1), :])
                nc.sync.dma_start(l_wv[:, 264 * k:264 * (k + 1)], w_v[128 * k:128 * (k + 1), :])
                nc.sync.dma_start(l_wp[:, 256 * k:256 * (k + 1)], w_p[128 * k:128 * (k + 1), :])
                nc.sync.dma_start(l_rgb[:, QCH * k:QCH * (k + 1)], rgb_s[128 * k:128 * (k + 1), :])
                nc.sync.dma_start(l_edge[k][:], edge[128 * k:128 * (k + 1), :])
            nc.sync.dma_start(l_id[:], ident[:])
            for x in range(2):
                nc.vector.tensor_copy(wq[x][:], l_wq[x][:])
                nc.vector.tensor_copy(wk[x][:], l_wk[x][:])
            nc.vector.tensor_copy(wv[:], l_wv[:])
            nc.vector.tensor_copy(wp[:], l_wp[:])
            nc.vector.tensor_copy(idn[:], l_id[:])
            nc.vector.tensor_copy(rgb_sb[:], l_rgb[:])
            for k in range(2):
                nc.vector.tensor_copy(edge_sb[k][:], l_edge[k][:])

            qq = [dp.tile([128, QCH], F32R, name=f"qq{x}", tag=f"qq{x}") for x in "AB"]
            kk = [dp.tile([128, N], F32R, name=f"kk{x}", tag=f"kk{x}") for x in "AB"]
            vto = dp.tile([128, NMT * 8 * HB], BF16, name="vto", tag="vto")
            outb = [dp.tile([128, QCH], F32, name=f"outb{k}", tag=f"outb{k}") for k in range(2)]

            # ---- pools (PSUM pools span the whole kernel: releasing a
            # PSUM pool makes successor tiles inherit released-zone deps
            # as multi-wait instructions, which walrus rejects) ----
            scp = tc.alloc_tile_pool(name="scp", bufs=2, space="PSUM")
            wps = tc.alloc_tile_pool(name="wps", bufs=4, space="PSUM")
            ep = tc.alloc_tile_pool(name="exp", bufs=4)
            sp = tc.alloc_tile_pool(name="stg", bufs=2)

            # ---- projections ----
            if True:
                pps = wps
                for x in range(2):          # qq (A/B head groups)
                    for j in range(2):
                        ps = pps.tile([128, 512], F32, name="ps_q", tag="w")
                        for k in range(2):
                            nc.tensor.matmul(
                                ps[:], wq[x][:, 128 * k:128 * (k + 1)],
                                rgb_sb[:, QCH * k + 512 * j:QCH * k + 512 * (j + 1)],
                                start=(k == 0), stop=(k == 1))
                        nc.vector.tensor_scalar_add(qq[x][:, 512 * j:512 * (j + 1)], ps[:], bq[x][:])
                for x in range(2):          # kk
                    for j in range(8):
                        ps = pps.tile([128, 512], F32, name="ps_k", tag="w")
                        for k in range(2):
                            nc.tensor.matmul(
                                ps[:], wk[x][:, 128 * k:128 * (k + 1)],
                                edge_sb[k][:, 512 * j:512 * (j + 1)],
                                start=(k == 0), stop=(k == 1))
                        nc.vector.tensor_scalar_add(kk[x][:, 512 * j:512 * (j + 1)], ps[:], bk[x][:])
                # v^T: vto[m, (h: 32 v | den)] per m-tile. wv has zero
                # columns at the denominator positions and bv carries the
                # 1.0s there, so no memset / strided writes are needed.
                for mt in range(NMT):
                    ps = pps.tile([128, 264], F32, name="ps_v", tag="w")
                    for k in range(2):
                        nc.tensor.matmul(
                            ps[:], edge_sb[k][:, 128 * mt:128 * (mt + 1)],
                            wv[:, 264 * k:264 * (k + 1)],
                            start=(k == 0), stop=(k == 1))
                    nc.vector.tensor_add(
                        vto[:, 8 * HB * mt:8 * HB * (mt + 1)], ps[:], bv[:])

            # ACT warmup: absorb the DVE tick (activation-bias const
            # tiles are DVE-written) so the first exp carries only the PE
            # wait — walrus allows a single wait per Activation.
            actw = dp.tile([1, 1], BF16, name="actw", tag="actw")
            nc.scalar.activation(actw[:], zrow[0:1, 0:1], AF.Copy)

            # ---- attention + output projection ----
            if True:
                for qc in range(2):
                    q0 = 512 * qc
                    av = []
                    for s in range(4):
                        a = wps.tile([128, 8 * HB], F32, name=f"av{s}", tag="w")
                        # one whole-bank start=True clear; the AV matmuls
                        # below all use start=False (first per-element write
                        # overwrites, later ones accumulate)
                        nc.tensor.matmul(a[:], zrow[:], vto[0:1, 0:8 * HB], start=True, stop=False)
                        av.append(a)
                    for x in range(2):
                        for pr in range(2):
                            # mtiles in pairs: PE switches tiling mode
                            # (32x128 QKT <-> 128x128 AV) once per pair
                            # instead of once per mtile (mode switch = PE
                            # drain)
                            for mt0 in range(0, NMT, 2):
                                ets = []
                                for mt in (mt0, mt0 + 1):
                                    sc = scp.tile([128, 1024], F32, name="sc", tag="sc")
                                    for j2 in range(2):
                                        j = 2 * pr + j2
                                        nc.tensor.matmul(
                                            sc[:, 512 * j2:512 * (j2 + 1)],
                                            kk[x][32 * j:32 * j + KD, 128 * mt:128 * (mt + 1)],
                                            qq[x][32 * j:32 * j + KD, q0:q0 + 512],
                                            start=True, stop=True,
                                            tile_position=(32 * j, 0))
                                    et = ep.tile([128, 1024], BF16, name="et", tag="et")
                                    nc.scalar.activation(et[:], sc[:], AF.Exp)
                                    ets.append(et)
                                for mt, et in zip((mt0, mt0 + 1), ets):
                                    for j2 in range(2):
                                        h = 4 * x + 2 * pr + j2
                                        for s in range(4):
                                            nc.tensor.matmul(
                                                av[s][:, HB * h:HB * (h + 1)],
                                                et[:, 512 * j2 + 128 * s:512 * j2 + 128 * (s + 1)],
                                                vto[:, 8 * HB * mt + HB * h:8 * HB * mt + HB * (h + 1)],
                                                start=False, stop=(mt == NMT - 1))
                    # normalize + relu + transpose + project
                    xxt = [sp.tile([128, 512], F32R, name=f"xxt{k}", tag=f"xxt{k}") for k in range(2)]
                    for s in range(4):
                        xxm = sp.tile([128, 8 * HB], F32, name="xxm", tag="xxm")
                        nc.vector.tensor_scalar_max(xxm[:], av[s][:], 0.0)
                        rden = sp.tile([128, 8], F32, name="rden", tag="rden")
                        nc.vector.reciprocal(
                            rden[:], xxm[:].rearrange("p (h x) -> p h x", x=HB)[:, :, 32])
                        xnm = sp.tile([128, 256], F32, name="xnm", tag="xnm")
                        for h in range(NUM_HEADS):
                            nc.vector.tensor_scalar_mul(
                                xnm[:, 32 * h:32 * (h + 1)], xxm[:, HB * h:HB * h + 32],
                                rden[:, h:h + 1])
                        for k in range(2):
                            tp = wps.tile([128, 128], F32, name="tp", tag="w")
                            nc.tensor.transpose(tp[:], xnm[:, 128 * k:128 * (k + 1)], idn[:])
                            nc.vector.tensor_copy(xxt[k][:, 128 * s:128 * (s + 1)], tp[:])
                    for ct in range(2):
                        ps = wps.tile([128, 512], F32, name="ps_p", tag="w")
                        for k in range(2):
                            nc.tensor.matmul(
                                ps[:], wp[:, 256 * k + 128 * ct:256 * k + 128 * (ct + 1)],
                                xxt[k][:], start=(k == 0), stop=(k == 1))
                        nc.vector.tensor_scalar_add(
                            outb[ct][:, q0:q0 + 512], ps[:], bp[:, ct:ct + 1])
            for ct in range(2):
                nc.gpsimd.dma_start(out[128 * ct:128 * (ct + 1), :], outb[ct][:])
            for _p in (sp, ep, wps, scp):
                _p.release()

    # walrus codegen accepts only ONE sync wait on compute instructions
    # (Matmult / Activation / TensorTensor / ...). The multi-wait cases
    # Tile emits here are all {self-engine, other}: a self-engine wait
    # orders an instruction against an earlier instruction on the SAME
    # in-order engine (WAW through PE's single PSUM write port, ACT/DVE
    # pipeline order), which the hardware already guarantees — drop it.
    _self_prefix = {
        "EngineType.PE": "PE",
        "EngineType.Activation": "Activation",
        "EngineType.DVE": "DVE",
        "EngineType.Pool": "Pool",
        "EngineType.SP": "SP",
    }
    for f in nc.m.functions:
        for bb in f.blocks:
            for inst in bb.instructions:
                si = inst.sync_info
                if si is None or not si.on_wait or len(si.on_wait) < 2:
                    continue
                pref = _self_prefix.get(str(getattr(inst, "engine", "")), None)
                if pref is None:
                    continue
                kept = [w for w in si.on_wait
                        if not str(w.ant_name).startswith(pref)]
                if not kept or len(kept) == len(si.on_wait):
                    continue
                si.on_wait = kept

    # Safety net: any instruction still carrying >1 wait gets all but its
    # last wait hoisted into preceding same-engine NoOps (1 wait each).
    uid = [0]
    for f in nc.m.functions:
        for bb in f.blocks:
            new_insts = []
            for inst in bb.instructions:
                si = inst.sync_info
                if si is not None and si.on_wait and len(si.on_wait) > 1:
                    for w in si.on_wait[:-1]:
                        uid[0] += 1
                        nop = mybir.InstNoOp(
                            name=f"I-waitsplit-{uid[0]}", ins=[], outs=[])
                        nop.engine = inst.engine
                        nop.sync_info = mybir.SyncInfo(
                            on_wait=[w], on_update=[])
                        new_insts.append(nop)
                    si.on_wait = [si.on_wait[-1]]
                new_insts.append(inst)
            bb.instructions = new_insts
    return nc


_CACHE = {}


def _prep_host(inputs):
    """Fold BN into weights; build head-split layouts shared by all cores."""
    f = np.float32
    Wq = (inputs["Wq"] * inputs["sq"][:, None]).astype(f)
    Wk = (inputs["Wk"] * inputs["sk"][:, None]).astype(f)
    Wv = (inputs["Wv"] * inputs["sv"][:, None]).astype(f)
    Wp = (inputs["Wp"] * inputs["sp"][:, None]).astype(f)

    def split(Wt, b):
        o = []
        for g in range(2):
            Wx = np.zeros((C, 128), f)
            bx = np.zeros((128, 1), f)
            for j in range(4):
                h = 4 * g + j
                Wx[:, 32 * j:32 * j + KD] = Wt[:, KD * h:KD * (h + 1)]
                bx[32 * j:32 * j + KD, 0] = b[KD * h:KD * (h + 1)]
            o.append((np.ascontiguousarray(Wx), bx))
        return o

    (wqA, bqA), (wqB, bqB) = split(Wq.T.astype(f), inputs["bq"])
    (wkA, bkA), (wkB, bkB) = split(Wk.T.astype(f), inputs["bk"])
    WvT = Wv.T.astype(f)                      # [C, 256] cols (h, d)
    wv_ext = np.zeros((C, 264), f)            # col 33h+32 stays 0
    bv_ext = np.zeros((264,), f)
    for h in range(NUM_HEADS):
        wv_ext[:, HB * h:HB * h + 32] = WvT[:, 32 * h:32 * (h + 1)]
        bv_ext[HB * h:HB * h + 32] = inputs["bv"][32 * h:32 * (h + 1)]
        bv_ext[HB * h + 32] = 1.0             # softmax denominator column
    return dict(
        w_qA=wqA, w_qB=wqB, w_kA=wkA, w_kB=wkB,
        w_v=wv_ext, w_p=np.ascontiguousarray(Wp.T),
        b_qA=bqA, b_qB=bqB, b_kA=bkA, b_kB=bkB,
        b_v=np.ascontiguousarray(np.broadcast_to(bv_ext, (128, 264))),
        b_p=inputs["bp"].astype(f).reshape(C, 1),
        ident=np.eye(128, dtype=f),
    )


def kernel(**inputs) -> np.ndarray:
    inputs = {k: np.asarray(v) for k, v in inputs.items()}
    if "nc" not in _CACHE:
        _CACHE["nc"] = build_nc()
    nc = _CACHE["nc"]

    shared = _prep_host(inputs)
    rgb = np.ascontiguousarray(inputs["rgb"].astype(np.float32).reshape(B, C, N))
    edge = np.ascontiguousarray(inputs["edge"].astype(np.float32).reshape(B, C, N))

    in_maps = []
    for core in range(8):
        b, qs = core // 4, core % 4
        m = dict(shared)
        m["rgb_s"] = np.ascontiguousarray(rgb[b][:, QCH * qs:QCH * (qs + 1)])
        m["edge"] = edge[b]
        in_maps.append(m)

    res = run_bass_kernel_spmd(nc, in_maps, core_ids=list(range(8)))
    full = np.zeros((B, C, N), np.float32)
    for core in range(8):
        b, qs = core // 4, core % 4
        full[b][:, QCH * qs:QCH * (qs + 1)] = res.results[core]["out"]
    return full.reshape(B, C, H, W)


# revision 23
# speedup vs baseline: 5089.3377x; 5089.3377x over previous
"""Cross-modal attention Trainium2 kernel.

Reference computation (all 1x1 convs + folded eval-mode BN):
  q = BN(Wq @ rgb), k = BN(Wk @ edge), v = BN(Wv @ edge)
  attn = softmax(q^T k) per head; xx = relu(attn @ v); out = BN(Wp @ xx)

Shapes: B=2, C=256, H=W=64 (N=4096), heads=8, key_dim=16, d=32.

Sharding: 8 cores = (batch b in {0,1}) x (query-slice qs in {0..3}, 1024
queries each). Each core computes K/V projections for the full N of its
batch (cheap) and attention + output projection for its query slice; the
host concatenates slices. No collectives.

Per-core dataflow (scores kept transposed so softmax-sum and the AV
contraction both run on the m axis without transposing the big matrix):
  sT[m, q] = sum_kd kk[kd, m] qq[kd, q]     (PE, fp32r, 32x128 row-tiled:
                                             2 heads concurrently)
  e = exp(sT)                               (ScalarE PSUM->SBUF bf16; max-
                                             subtraction skipped: |s|<~45)
  av[q, (h: d|den)] += e[mtile]^T @ [v|1]   (PE bf16, exp as stationary)
  xx = relu(av) * recip(den)                (VectorE, per-partition scalar)
  out = Wp^T @ xx^T + bp                    (PE transpose + fp32r matmul)
"""

import sys

for p in ("/opt/trn_rl_repo", "/opt/trn_rl_repo/concourse"):
    if p not in sys.path:
        sys.path.insert(0, p)

import numpy as np

import concourse.bass as bass
import concourse.mybir as mybir
import concourse.tile as tile
from concourse.bass_utils import run_bass_kernel_spmd
from concourse.tile_rust import add_dep_helper

F32 = mybir.dt.float32
F32R = mybir.dt.float32r
BF16 = mybir.dt.bfloat16
AF = mybir.ActivationFunctionType

NUM_HEADS, KD, D = 8, 16, 32
B, C, H, W = 2, 256, 64, 64
N = H * W            # 4096 keys per batch
QCH = 1024           # queries per core
NMT = N // 128       # 32 m-tiles
HB = 33              # per-head AV block: 32 v-cols + 1 denominator col


def build_nc(trace_scopes=False):
    nc = bass.Bass()

    rgb_s = nc.dram_tensor("rgb_s", [C, QCH], F32R, kind="ExternalInput")
    edge = nc.dram_tensor("edge", [C, N], F32R, kind="ExternalInput")
    w_qA = nc.dram_tensor("w_qA", [C, 128], F32R, kind="ExternalInput")
    w_qB = nc.dram_tensor("w_qB", [C, 128], F32R, kind="ExternalInput")
    w_kA = nc.dram_tensor("w_kA", [C, 128], F32R, kind="ExternalInput")
    w_kB = nc.dram_tensor("w_kB", [C, 128], F32R, kind="ExternalInput")
    w_v = nc.dram_tensor("w_v", [C, 264], F32R, kind="ExternalInput")
    w_p = nc.dram_tensor("w_p", [256, C], F32R, kind="ExternalInput")
    b_qA = nc.dram_tensor("b_qA", [128, 1], F32, kind="ExternalInput")
    b_qB = nc.dram_tensor("b_qB", [128, 1], F32, kind="ExternalInput")
    b_kA = nc.dram_tensor("b_kA", [128, 1], F32, kind="ExternalInput")
    b_kB = nc.dram_tensor("b_kB", [128, 1], F32, kind="ExternalInput")
    b_v = nc.dram_tensor("b_v", [128, 264], F32, kind="ExternalInput")
    b_p = nc.dram_tensor("b_p", [C, 1], F32, kind="ExternalInput")
    ident = nc.dram_tensor("ident", [128, 128], F32, kind="ExternalInput")
    out = nc.dram_tensor("out", [C, QCH], F32, kind="ExternalOutput")

    with tile.TileContext(nc) as tc:
        with tc.tile_pool(name="const", bufs=1) as cp, \
             tc.tile_pool(name="data", bufs=1) as dp:
            wq = [cp.tile([128, 256], F32R, name=f"wq{x}", tag=f"wq{x}") for x in "AB"]
            wk = [cp.tile([128, 256], F32R, name=f"wk{x}", tag=f"wk{x}") for x in "AB"]
            wv = cp.tile([128, 528], F32R, name="wv", tag="wv")
            wp = cp.tile([128, 512], F32R, name="wp", tag="wp")
            bq = [cp.tile([128, 1], F32, name=f"bq{x}", tag=f"bq{x}") for x in "AB"]
            bk = [cp.tile([128, 1], F32, name=f"bk{x}", tag=f"bk{x}") for x in "AB"]
            bv = cp.tile([128, 264], F32, name="bv", tag="bv")
            bp = cp.tile([128, 2], F32, name="bp", tag="bp")
            idn = cp.tile([128, 128], F32, name="idn", tag="idn")
            zrow = cp.tile([1, 128], BF16, name="zrow", tag="zrow")

            nc.sync.dma_start(bp[:, 0:1], b_p[0:128, :])
            nc.sync.dma_start(bp[:, 1:2], b_p[128:256, :])
            nc.sync.dma_start(bq[0][:], b_qA[:])
            nc.sync.dma_start(bq[1][:], b_qB[:])
            nc.sync.dma_start(bk[0][:], b_kA[:])
            nc.sync.dma_start(bk[1][:], b_kB[:])
            nc.sync.dma_start(bv[:], b_v[:])
            nc.vector.memset(zrow[:], 0.0)

            rgb_sb = dp.tile([128, 2 * QCH], F32R, name="rgb_sb", tag="rgb")
            edge_sb = [dp.tile([128, N], F32R, name=f"edge{k}", tag=f"edge{k}") for k in range(2)]

            # Direct DMA into the f32r tiles (f32r bits == f32; the HW
            # matmul rounds on read). Matmuls that end up with multiple
            # DMA-lane waits are legalized by the wait-splitter post-pass.
            for k in range(2):
                nc.sync.dma_start(wq[0][:, 128 * k:128 * (k + 1)], w_qA[128 * k:128 * (k + 1), :])
                nc.sync.dma_start(wq[1][:, 128 * k:128 * (k + 1)], w_qB[128 * k:128 * (k + 1), :])
                nc.sync.dma_start(wk[0][:, 128 * k:128 * (k + 1)], w_kA[128 * k:128 * (k + 1), :])
                nc.sync.dma_start(wk[1][:, 128 * k:128 * (k + 1)], w_kB[128 * k:128 * (k + 1), :])
                nc.sync.dma_start(wv[:, 264 * k:264 * (k + 1)], w_v[128 * k:128 * (k + 1), :])
                nc.sync.dma_start(wp[:, 256 * k:256 * (k + 1)], w_p[128 * k:128 * (k + 1), :])
                nc.sync.dma_start(rgb_sb[:, QCH * k:QCH * (k + 1)], rgb_s[128 * k:128 * (k + 1), :])
                nc.sync.dma_start(edge_sb[k][:], edge[128 * k:128 * (k + 1), :])
            nc.sync.dma_start(idn[:], ident[:])

            qq = [dp.tile([128, QCH], F32R, name=f"qq{x}", tag=f"qq{x}") for x in "AB"]
            kk = [dp.tile([128, N], F32R, name=f"kk{x}", tag=f"kk{x}") for x in "AB"]
            vto = dp.tile([128, NMT * 8 * HB], BF16, name="vto", tag="vto")
            outb = [dp.tile([128, QCH], F32, name=f"outb{k}", tag=f"outb{k}") for k in range(2)]

            # ---- pools (PSUM pools span the whole kernel: releasing a
            # PSUM pool makes successor tiles inherit released-zone deps
            # as multi-wait instructions, which walrus rejects) ----
            scp = tc.alloc_tile_pool(name="scp", bufs=2, space="PSUM")
            wps = tc.alloc_tile_pool(name="wps", bufs=4, space="PSUM")
            ep = tc.alloc_tile_pool(name="exp", bufs=4)
            sp = tc.alloc_tile_pool(name="stg", bufs=2)

            # ---- projections ----
            if True:
                pps = wps
                for x in range(2):          # qq (A/B head groups)
                    for j in range(2):
                        ps = pps.tile([128, 512], F32, name="ps_q", tag="w")
                        for k in range(2):
                            nc.tensor.matmul(
                                ps[:], wq[x][:, 128 * k:128 * (k + 1)],
                                rgb_sb[:, QCH * k + 512 * j:QCH * k + 512 * (j + 1)],
                                start=(k == 0), stop=(k == 1))
                        nc.vector.tensor_scalar_add(qq[x][:, 512 * j:512 * (j + 1)], ps[:], bq[x][:])
                for x in range(2):          # kk
                    for j in range(8):
                        ps = pps.tile([128, 512], F32, name="ps_k", tag="w")
                        for k in range(2):
                            nc.tensor.matmul(
                                ps[:], wk[x][:, 128 * k:128 * (k + 1)],
                                edge_sb[k][:, 512 * j:512 * (j + 1)],
                                start=(k == 0), stop=(k == 1))
                        nc.vector.tensor_scalar_add(kk[x][:, 512 * j:512 * (j + 1)], ps[:], bk[x][:])
                # v^T: vto[m, (h: 32 v | den)] per m-tile. wv has zero
                # columns at the denominator positions and bv carries the
                # 1.0s there, so no memset / strided writes are needed.
                for mt in range(NMT):
                    ps = pps.tile([128, 264], F32, name="ps_v", tag="w")
                    for k in range(2):
                        nc.tensor.matmul(
                            ps[:], edge_sb[k][:, 128 * mt:128 * (mt + 1)],
                            wv[:, 264 * k:264 * (k + 1)],
                            start=(k == 0), stop=(k == 1))
                    nc.vector.tensor_add(
                        vto[:, 8 * HB * mt:8 * HB * (mt + 1)], ps[:], bv[:])

            # ACT warmup: absorb the DVE tick (activation-bias const
            # tiles are DVE-written) so the first exp carries only the PE
            # wait — walrus allows a single wait per Activation.
            actw = dp.tile([1, 1], BF16, name="actw", tag="actw")
            nc.scalar.activation(actw[:], zrow[0:1, 0:1], AF.Exp)

            # ---- attention + output projection ----
            if True:
                for qc in range(2):
                    q0 = 512 * qc
                    av = []
                    for s in range(4):
                        a = wps.tile([128, 8 * HB], F32, name=f"av{s}", tag="w")
                        # one whole-bank start=True clear; the AV matmuls
                        # below all use start=False (first per-element write
                        # overwrites, later ones accumulate)
                        nc.tensor.matmul(a[:], zrow[:], vto[0:1, 0:8 * HB], start=True, stop=False)
                        av.append(a)
                    for x in range(2):
                        for pr in range(2):
                            # mtiles in pairs: PE switches tiling mode
                            # (32x128 QKT <-> 128x128 AV) once per pair
                            # instead of once per mtile (mode switch = PE
                            # drain)
                            for mt0 in range(0, NMT, 2):
                                ets = []
                                for mt in (mt0, mt0 + 1):
                                    sc = scp.tile([128, 1024], F32, name="sc", tag="sc")
                                    for j2 in range(2):
                                        j = 2 * pr + j2
                                        nc.tensor.matmul(
                                            sc[:, 512 * j2:512 * (j2 + 1)],
                                            kk[x][32 * j:32 * j + KD, 128 * mt:128 * (mt + 1)],
                                            qq[x][32 * j:32 * j + KD, q0:q0 + 512],
                                            start=True, stop=True,
                                            tile_position=(32 * j, 0))
                                    et = ep.tile([128, 1024], BF16, name="et", tag="et")
                                    nc.scalar.activation(et[:], sc[:], AF.Exp)
                                    ets.append(et)
                                for mt, et in zip((mt0, mt0 + 1), ets):
                                    for j2 in range(2):
                                        h = 4 * x + 2 * pr + j2
                                        for s in range(4):
                                            nc.tensor.matmul(
                                                av[s][:, HB * h:HB * (h + 1)],
                                                et[:, 512 * j2 + 128 * s:512 * j2 + 128 * (s + 1)],
                                                vto[:, 8 * HB * mt + HB * h:8 * HB * mt + HB * (h + 1)],
                                                start=False, stop=(mt == NMT - 1))
                    # normalize + relu + transpose + project
                    xxt = [sp.tile([128, 512], F32R, name=f"xxt{k}", tag=f"xxt{k}") for k in range(2)]
                    for s in range(4):
                        xxm = sp.tile([128, 8 * HB], F32, name="xxm", tag="xxm")
                        nc.vector.tensor_scalar_max(xxm[:], av[s][:], 0.0)
                        rden = sp.tile([128, 8], F32, name="rden", tag="rden")
                        nc.vector.reciprocal(
                            rden[:], xxm[:].rearrange("p (h x) -> p h x", x=HB)[:, :, 32])
                        xnm = sp.tile([128, 256], F32, name="xnm", tag="xnm")
                        for h in range(NUM_HEADS):
                            nc.vector.tensor_scalar_mul(
                                xnm[:, 32 * h:32 * (h + 1)], xxm[:, HB * h:HB * h + 32],
                                rden[:, h:h + 1])
                        for k in range(2):
                            tp = wps.tile([128, 128], F32, name="tp", tag="w")
                            nc.tensor.transpose(tp[:], xnm[:, 128 * k:128 * (k + 1)], idn[:])
                            nc.vector.tensor_copy(xxt[k][:, 128 * s:128 * (s + 1)], tp[:])
                    for ct in range(2):
                        ps = wps.tile([128, 512], F32, name="ps_p", tag="w")
                        for k in range(2):
                            nc.tensor.matmul(
                                ps[:], wp[:, 256 * k + 128 * ct:256 * k + 128 * (ct + 1)],
                                xxt[k][:], start=(k == 0), stop=(k == 1))
                        nc.vector.tensor_scalar_add(
                            outb[ct][:, q0:q0 + 512], ps[:], bp[:, ct:ct + 1])
            for ct in range(2):
                nc.gpsimd.dma_start(out[128 * ct:128 * (ct + 1), :], outb[ct][:])
            for _p in (sp, ep, wps, scp):
                _p.release()

    # walrus codegen accepts only ONE sync wait on compute instructions
    # (Matmult / Activation / TensorTensor / ...). The multi-wait cases
    # Tile emits here are all {self-engine, other}: a self-engine wait
    # orders an instruction against an earlier instruction on the SAME
    # in-order engine (WAW through PE's single PSUM write port, ACT/DVE
    # pipeline order), which the hardware already guarantees — drop it.
    _self_prefix = {
        "EngineType.PE": "PE",
        "EngineType.Activation": "Activation",
        "EngineType.DVE": "DVE",
        "EngineType.Pool": "Pool",
        "EngineType.SP": "SP",
    }
    for f in nc.m.functions:
        for bb in f.blocks:
            for inst in bb.instructions:
                si = inst.sync_info
                if si is None or not si.on_wait or len(si.on_wait) < 2:
                    continue
                pref = _self_prefix.get(str(getattr(inst, "engine", "")), None)
                if pref is None:
                    continue
                kept = [w for w in si.on_wait
                        if not str(w.ant_name).startswith(pref)]
                if not kept or len(kept) == len(si.on_wait):
                    continue
                si.on_wait = kept

    # Safety net: any instruction still carrying >1 wait gets all but its
    # last wait hoisted into preceding same-engine NoOps (1 wait each).
    uid = [0]
    for f in nc.m.functions:
        for bb in f.blocks:
            new_insts = []
            for inst in bb.instructions:
                si = inst.sync_info
                if si is not None and si.on_wait and len(si.on_wait) > 1:
                    for w in si.on_wait[:-1]:
                        uid[0] += 1
                        nop = mybir.InstNoOp(
                            name=f"I-waitsplit-{uid[0]}", ins=[], outs=[])
                        nop.engine = inst.engine
                        nop.sync_info = mybir.SyncInfo(
                            on_wait=[w], on_update=[])
                        new_insts.append(nop)
                    si.on_wait = [si.on_wait[-1]]
                new_insts.append(inst)
            bb.instructions = new_insts
    return nc


_CACHE = {}


def _prep_host(inputs):
    """Fold BN into weights; build head-split layouts shared by all cores."""
    f = np.float32
    Wq = (inputs["Wq"] * inputs["sq"][:, None]).astype(f)
    Wk = (inputs["Wk"] * inputs["sk"][:, None]).astype(f)
    Wv = (inputs["Wv"] * inputs["sv"][:, None]).astype(f)
    Wp = (inputs["Wp"] * inputs["sp"][:, None]).astype(f)

    def split(Wt, b):
        o = []
        for g in range(2):
            Wx = np.zeros((C, 128), f)
            bx = np.zeros((128, 1), f)
            for j in range(4):
                h = 4 * g + j
                Wx[:, 32 * j:32 * j + KD] = Wt[:, KD * h:KD * (h + 1)]
                bx[32 * j:32 * j + KD, 0] = b[KD * h:KD * (h + 1)]
            o.append((np.ascontiguousarray(Wx), bx))
        return o

    (wqA, bqA), (wqB, bqB) = split(Wq.T.astype(f), inputs["bq"])
    (wkA, bkA), (wkB, bkB) = split(Wk.T.astype(f), inputs["bk"])
    WvT = Wv.T.astype(f)                      # [C, 256] cols (h, d)
    wv_ext = np.zeros((C, 264), f)            # col 33h+32 stays 0
    bv_ext = np.zeros((264,), f)
    for h in range(NUM_HEADS):
        wv_ext[:, HB * h:HB * h + 32] = WvT[:, 32 * h:32 * (h + 1)]
        bv_ext[HB * h:HB * h + 32] = inputs["bv"][32 * h:32 * (h + 1)]
        bv_ext[HB * h + 32] = 1.0             # softmax denominator column
    return dict(
        w_qA=wqA, w_qB=wqB, w_kA=wkA, w_kB=wkB,
        w_v=wv_ext, w_p=np.ascontiguousarray(Wp.T),
        b_qA=bqA, b_qB=bqB, b_kA=bkA, b_kB=bkB,
        b_v=np.ascontiguousarray(np.broadcast_to(bv_ext, (128, 264))),
        b_p=inputs["bp"].astype(f).reshape(C, 1),
        ident=np.eye(128, dtype=f),
    )


def kernel(**inputs) -> np.ndarray:
    inputs = {k: np.asarray(v) for k, v in inputs.items()}
    if "nc" not in _CACHE:
        _CACHE["nc"] = build_nc()
    nc = _CACHE["nc"]

    shared = _prep_host(inputs)
    rgb = np.ascontiguousarray(inputs["rgb"].astype(np.float32).reshape(B, C, N))
    edge = np.ascontiguousarray(inputs["edge"].astype(np.float32).reshape(B, C, N))

    in_maps = []
    for core in range(8):
        b, qs = core // 4, core % 4
        m = dict(shared)
        m["rgb_s"] = np.ascontiguousarray(rgb[b][:, QCH * qs:QCH * (qs + 1)])
        m["edge"] = edge[b]
        in_maps.append(m)

    res = run_bass_kernel_spmd(nc, in_maps, core_ids=list(range(8)))
    full = np.zeros((B, C, N), np.float32)
    for core in range(8):
        b, qs = core // 4, core % 4
        full[b][:, QCH * qs:QCH * (qs + 1)] = res.results[core]["out"]
    return full.reshape(B, C, H, W)


# revision 26
# speedup vs baseline: 5353.3937x; 1.0519x over previous
"""Cross-modal attention Trainium2 kernel.

Reference computation (all 1x1 convs + folded eval-mode BN):
  q = BN(Wq @ rgb), k = BN(Wk @ edge), v = BN(Wv @ edge)
  attn = softmax(q^T k) per head; xx = relu(attn @ v); out = BN(Wp @ xx)

Shapes: B=2, C=256, H=W=64 (N=4096), heads=8, key_dim=16, d=32.

Sharding: 8 cores = (batch b in {0,1}) x (query-slice qs in {0..3}, 1024
queries each). Each core computes K/V projections for the full N of its
batch (cheap) and attention + output projection for its query slice; the
host concatenates slices. No collectives.

Per-core dataflow (scores kept transposed so softmax-sum and the AV
contraction both run on the m axis without transposing the big matrix):
  sT[m, q] = sum_kd kk[kd, m] qq[kd, q]     (PE, fp32r, 32x128 row-tiled:
                                             2 heads concurrently)
  e = exp(sT)                               (ScalarE PSUM->SBUF bf16; max-
                                             subtraction skipped: |s|<~45)
  av[q, (h: d|den)] += e[mtile]^T @ [v|1]   (PE bf16, exp as stationary)
  xx = relu(av) * recip(den)                (VectorE, per-partition scalar)
  out = Wp^T @ xx^T + bp                    (PE transpose + fp32r matmul)
"""

import sys

for p in ("/opt/trn_rl_repo", "/opt/trn_rl_repo/concourse"):
    if p not in sys.path:
        sys.path.insert(0, p)

import numpy as np

import concourse.bass as bass
import concourse.mybir as mybir
import concourse.tile as tile
from concourse.bass_utils import run_bass_kernel_spmd
from concourse.tile_rust import add_dep_helper

F32 = mybir.dt.float32
F32R = mybir.dt.float32r
BF16 = mybir.dt.bfloat16
AF = mybir.ActivationFunctionType

NUM_HEADS, KD, D = 8, 16, 32
B, C, H, W = 2, 256, 64, 64
N = H * W            # 4096 keys per batch
QCH = 1024           # queries per core
NMT = N // 128       # 32 m-tiles
HB = 33              # per-head AV block: 32 v-cols + 1 denominator col


def build_nc(trace_scopes=False):
    nc = bass.Bass()

    rgb_s = nc.dram_tensor("rgb_s", [C, QCH], F32R, kind="ExternalInput")
    edge = nc.dram_tensor("edge", [C, N], F32R, kind="ExternalInput")
    w_qA = nc.dram_tensor("w_qA", [C, 128], F32R, kind="ExternalInput")
    w_qB = nc.dram_tensor("w_qB", [C, 128], F32R, kind="ExternalInput")
    w_kA = nc.dram_tensor("w_kA", [C, 128], F32R, kind="ExternalInput")
    w_kB = nc.dram_tensor("w_kB", [C, 128], F32R, kind="ExternalInput")
    w_v = nc.dram_tensor("w_v", [C, 264], F32R, kind="ExternalInput")
    w_p = nc.dram_tensor("w_p", [256, C], F32R, kind="ExternalInput")
    b_qA = nc.dram_tensor("b_qA", [128, 1], F32, kind="ExternalInput")
    b_qB = nc.dram_tensor("b_qB", [128, 1], F32, kind="ExternalInput")
    b_kA = nc.dram_tensor("b_kA", [128, 1], F32, kind="ExternalInput")
    b_kB = nc.dram_tensor("b_kB", [128, 1], F32, kind="ExternalInput")
    b_v = nc.dram_tensor("b_v", [128, 264], F32, kind="ExternalInput")
    b_p = nc.dram_tensor("b_p", [C, 1], F32, kind="ExternalInput")
    ident = nc.dram_tensor("ident", [128, 128], F32, kind="ExternalInput")
    out = nc.dram_tensor("out", [C, QCH], F32, kind="ExternalOutput")

    with tile.TileContext(nc) as tc:
        with tc.tile_pool(name="const", bufs=1) as cp, \
             tc.tile_pool(name="data", bufs=1) as dp:
            wq = [cp.tile([128, 256], F32R, name=f"wq{x}", tag=f"wq{x}") for x in "AB"]
            wk = [cp.tile([128, 256], F32R, name=f"wk{x}", tag=f"wk{x}") for x in "AB"]
            wv = cp.tile([128, 528], F32R, name="wv", tag="wv")
            wp = cp.tile([128, 512], F32R, name="wp", tag="wp")
            bq = [cp.tile([128, 1], F32, name=f"bq{x}", tag=f"bq{x}") for x in "AB"]
            bk = [cp.tile([128, 1], F32, name=f"bk{x}", tag=f"bk{x}") for x in "AB"]
            bv = cp.tile([128, 264], F32, name="bv", tag="bv")
            bp = cp.tile([128, 2], F32, name="bp", tag="bp")
            idn = cp.tile([128, 128], F32, name="idn", tag="idn")
            zrow = cp.tile([1, 128], BF16, name="zrow", tag="zrow")

            nc.sync.dma_start(bp[:, 0:1], b_p[0:128, :])
            nc.sync.dma_start(bp[:, 1:2], b_p[128:256, :])
            nc.sync.dma_start(bq[0][:], b_qA[:])
            nc.sync.dma_start(bq[1][:], b_qB[:])
            nc.sync.dma_start(bk[0][:], b_kA[:])
            nc.sync.dma_start(bk[1][:], b_kB[:])
            nc.sync.dma_start(bv[:], b_v[:])
            nc.vector.memset(zrow[:], 0.0)

            rgb_sb = dp.tile([128, 2 * QCH], F32R, name="rgb_sb", tag="rgb")
            edge_sb = [dp.tile([128, N], F32R, name=f"edge{k}", tag=f"edge{k}") for k in range(2)]

            # Direct DMA into the f32r tiles (f32r bits == f32; the HW
            # matmul rounds on read). Matmuls that end up with multiple
            # DMA-lane waits are legalized by the wait-splitter post-pass.
            for k in range(2):
                nc.sync.dma_start(wq[0][:, 128 * k:128 * (k + 1)], w_qA[128 * k:128 * (k + 1), :])
                nc.sync.dma_start(wq[1][:, 128 * k:128 * (k + 1)], w_qB[128 * k:128 * (k + 1), :])
                nc.sync.dma_start(wk[0][:, 128 * k:128 * (k + 1)], w_kA[128 * k:128 * (k + 1), :])
                nc.sync.dma_start(wk[1][:, 128 * k:128 * (k + 1)], w_kB[128 * k:128 * (k + 1), :])
                nc.sync.dma_start(wv[:, 264 * k:264 * (k + 1)], w_v[128 * k:128 * (k + 1), :])
                nc.sync.dma_start(wp[:, 256 * k:256 * (k + 1)], w_p[128 * k:128 * (k + 1), :])
                nc.sync.dma_start(rgb_sb[:, QCH * k:QCH * (k + 1)], rgb_s[128 * k:128 * (k + 1), :])
                nc.sync.dma_start(edge_sb[k][:], edge[128 * k:128 * (k + 1), :])
            nc.sync.dma_start(idn[:], ident[:])

            qq = [dp.tile([128, QCH], F32R, name=f"qq{x}", tag=f"qq{x}") for x in "AB"]
            kk = [dp.tile([128, N], F32R, name=f"kk{x}", tag=f"kk{x}") for x in "AB"]
            vto = dp.tile([128, NMT * 8 * HB], BF16, name="vto", tag="vto")
            outb = [dp.tile([128, QCH], F32, name=f"outb{k}", tag=f"outb{k}") for k in range(2)]

            # ---- pools (PSUM pools span the whole kernel: releasing a
            # PSUM pool makes successor tiles inherit released-zone deps
            # as multi-wait instructions, which walrus rejects) ----
            scp = tc.alloc_tile_pool(name="scp", bufs=2, space="PSUM")
            wps = tc.alloc_tile_pool(name="wps", bufs=4, space="PSUM")
            ep = tc.alloc_tile_pool(name="exp", bufs=16)
            sp = tc.alloc_tile_pool(name="stg", bufs=6)

            # ---- projections ----
            if True:
                pps = wps
                for x in range(2):          # qq (A/B head groups)
                    for j in range(2):
                        ps = pps.tile([128, 512], F32, name="ps_q", tag="w")
                        for k in range(2):
                            nc.tensor.matmul(
                                ps[:], wq[x][:, 128 * k:128 * (k + 1)],
                                rgb_sb[:, QCH * k + 512 * j:QCH * k + 512 * (j + 1)],
                                start=(k == 0), stop=(k == 1))
                        nc.vector.tensor_scalar_add(qq[x][:, 512 * j:512 * (j + 1)], ps[:], bq[x][:])
                for x in range(2):          # kk
                    for j in range(8):
                        ps = pps.tile([128, 512], F32, name="ps_k", tag="w")
                        for k in range(2):
                            nc.tensor.matmul(
                                ps[:], wk[x][:, 128 * k:128 * (k + 1)],
                                edge_sb[k][:, 512 * j:512 * (j + 1)],
                                start=(k == 0), stop=(k == 1))
                        nc.vector.tensor_scalar_add(kk[x][:, 512 * j:512 * (j + 1)], ps[:], bk[x][:])
                # v^T: vto[m, (h: 32 v | den)] per m-tile. wv has zero
                # columns at the denominator positions and bv carries the
                # 1.0s there, so no memset / strided writes are needed.
                for mt in range(NMT):
                    ps = pps.tile([128, 264], F32, name="ps_v", tag="w")
                    for k in range(2):
                        nc.tensor.matmul(
                            ps[:], edge_sb[k][:, 128 * mt:128 * (mt + 1)],
                            wv[:, 264 * k:264 * (k + 1)],
                            start=(k == 0), stop=(k == 1))
                    nc.vector.tensor_add(
                        vto[:, 8 * HB * mt:8 * HB * (mt + 1)], ps[:], bv[:])

            # ACT warmup: absorb the DVE tick (activation-bias const
            # tiles are DVE-written) so the first exp carries only the PE
            # wait — walrus allows a single wait per Activation.
            actw = dp.tile([1, 1], BF16, name="actw", tag="actw")
            nc.scalar.activation(actw[:], zrow[0:1, 0:1], AF.Exp)

            # ---- attention + output projection ----
            if True:
                for qc in range(2):
                    q0 = 512 * qc
                    av = []
                    for s in range(4):
                        a = wps.tile([128, 8 * HB], F32, name=f"av{s}", tag="w")
                        # one whole-bank start=True clear; the AV matmuls
                        # below all use start=False (first per-element write
                        # overwrites, later ones accumulate)
                        nc.tensor.matmul(a[:], zrow[:], vto[0:1, 0:8 * HB], start=True, stop=False)
                        av.append(a)
                    for x in range(2):
                        for pr in range(2):
                            # mtiles in pairs: PE switches tiling mode
                            # (32x128 QKT <-> 128x128 AV) once per pair
                            # instead of once per mtile (mode switch = PE
                            # drain)
                            for mt0 in range(0, NMT, 2):
                                ets = []
                                for mt in (mt0, mt0 + 1):
                                    sc = scp.tile([128, 1024], F32, name="sc", tag="sc")
                                    for j2 in range(2):
                                        j = 2 * pr + j2
                                        nc.tensor.matmul(
                                            sc[:, 512 * j2:512 * (j2 + 1)],
                                            kk[x][32 * j:32 * j + KD, 128 * mt:128 * (mt + 1)],
                                            qq[x][32 * j:32 * j + KD, q0:q0 + 512],
                                            start=True, stop=True,
                                            tile_position=(32 * j, 0))
                                    et = ep.tile([128, 1024], BF16, name="et", tag="et")
                                    nc.scalar.activation(et[:], sc[:], AF.Exp)
                                    ets.append(et)
                                for mt, et in zip((mt0, mt0 + 1), ets):
                                    for j2 in range(2):
                                        h = 4 * x + 2 * pr + j2
                                        for s in range(4):
                                            nc.tensor.matmul(
                                                av[s][:, HB * h:HB * (h + 1)],
                                                et[:, 512 * j2 + 128 * s:512 * j2 + 128 * (s + 1)],
                                                vto[:, 8 * HB * mt + HB * h:8 * HB * mt + HB * (h + 1)],
                                                start=False, stop=(mt == NMT - 1))
                    # normalize + relu + transpose + project
                    xxt = [sp.tile([128, 512], F32R, name=f"xxt{k}", tag=f"xxt{k}") for k in range(2)]
                    for s in range(4):
                        xxm = sp.tile([128, 8 * HB], F32, name="xxm", tag="xxm")
                        nc.vector.tensor_scalar_max(xxm[:], av[s][:], 0.0)
                        rden = sp.tile([128, 8], F32, name="rden", tag="rden")
                        nc.vector.reciprocal(
                            rden[:], xxm[:].rearrange("p (h x) -> p h x", x=HB)[:, :, 32])
                        xnm = sp.tile([128, 256], F32, name="xnm", tag="xnm")
                        for h in range(NUM_HEADS):
                            nc.vector.tensor_scalar_mul(
                                xnm[:, 32 * h:32 * (h + 1)], xxm[:, HB * h:HB * h + 32],
                                rden[:, h:h + 1])
                        for k in range(2):
                            tp = wps.tile([128, 128], F32, name="tp", tag="w")
                            nc.tensor.transpose(tp[:], xnm[:, 128 * k:128 * (k + 1)], idn[:])
                            nc.vector.tensor_copy(xxt[k][:, 128 * s:128 * (s + 1)], tp[:])
                    for ct in range(2):
                        ps = wps.tile([128, 512], F32, name="ps_p", tag="w")
                        for k in range(2):
                            nc.tensor.matmul(
                                ps[:], wp[:, 256 * k + 128 * ct:256 * k + 128 * (ct + 1)],
                                xxt[k][:], start=(k == 0), stop=(k == 1))
                        nc.vector.tensor_scalar_add(
                            outb[ct][:, q0:q0 + 512], ps[:], bp[:, ct:ct + 1])
            for ct in range(2):
                nc.gpsimd.dma_start(out[128 * ct:128 * (ct + 1), :], outb[ct][:])
            for _p in (sp, ep, wps, scp):
                _p.release()

    # walrus codegen accepts only ONE sync wait on compute instructions
    # (Matmult / Activation / TensorTensor / ...). The multi-wait cases
    # Tile emits here are all {self-engine, other}: a self-engine wait
    # orders an instruction against an earlier instruction on the SAME
    # in-order engine (WAW through PE's single PSUM write port, ACT/DVE
    # pipeline order), which the hardware already guarantees — drop it.
    _self_prefix = {
        "EngineType.PE": "PE",
        "EngineType.Activation": "Activation",
        "EngineType.DVE": "DVE",
        "EngineType.Pool": "Pool",
        "EngineType.SP": "SP",
    }
    for f in nc.m.functions:
        for bb in f.blocks:
            for inst in bb.instructions:
                si = inst.sync_info
                if si is None or not si.on_wait or len(si.on_wait) < 2:
                    continue
                pref = _self_prefix.get(str(getattr(inst, "engine", "")), None)
                if pref is None:
                    continue
                kept = [w for w in si.on_wait
                        if not str(w.ant_name).startswith(pref)]
                if not kept or len(kept) == len(si.on_wait):
                    continue
                si.on_wait = kept

    # Safety net: any instruction still carrying >1 wait gets all but its
    # last wait hoisted into preceding same-engine NoOps (1 wait each).
    uid = [0]
    for f in nc.m.functions:
        for bb in f.blocks:
            new_insts = []
            for inst in bb.instructions:
                si = inst.sync_info
                if si is not None and si.on_wait and len(si.on_wait) > 1:
                    for w in si.on_wait[:-1]:
                        uid[0] += 1
                        nop = mybir.InstNoOp(
                            name=f"I-waitsplit-{uid[0]}", ins=[], outs=[])
                        nop.engine = inst.engine
                        nop.sync_info = mybir.SyncInfo(
                            on_wait=[w], on_update=[])
                        new_insts.append(nop)
                    si.on_wait = [si.on_wait[-1]]
                new_insts.append(inst)
            bb.instructions = new_insts
    return nc


_CACHE = {}


def _prep_host(inputs):
    """Fold BN into weights; build head-split layouts shared by all cores."""
    f = np.float32
    Wq = (inputs["Wq"] * inputs["sq"][:, None]).astype(f)
    Wk = (inputs["Wk"] * inputs["sk"][:, None]).astype(f)
    Wv = (inputs["Wv"] * inputs["sv"][:, None]).astype(f)
    Wp = (inputs["Wp"] * inputs["sp"][:, None]).astype(f)

    def split(Wt, b):
        o = []
        for g in range(2):
            Wx = np.zeros((C, 128), f)
            bx = np.zeros((128, 1), f)
            for j in range(4):
                h = 4 * g + j
                Wx[:, 32 * j:32 * j + KD] = Wt[:, KD * h:KD * (h + 1)]
                bx[32 * j:32 * j + KD, 0] = b[KD * h:KD * (h + 1)]
            o.append((np.ascontiguousarray(Wx), bx))
        return o

    (wqA, bqA), (wqB, bqB) = split(Wq.T.astype(f), inputs["bq"])
    (wkA, bkA), (wkB, bkB) = split(Wk.T.astype(f), inputs["bk"])
    WvT = Wv.T.astype(f)                      # [C, 256] cols (h, d)
    wv_ext = np.zeros((C, 264), f)            # col 33h+32 stays 0
    bv_ext = np.zeros((264,), f)
    for h in range(NUM_HEADS):
        wv_ext[:, HB * h:HB * h + 32] = WvT[:, 32 * h:32 * (h + 1)]
        bv_ext[HB * h:HB * h + 32] = inputs["bv"][32 * h:32 * (h + 1)]
        bv_ext[HB * h + 32] = 1.0             # softmax denominator column
    return dict(
        w_qA=wqA, w_qB=wqB, w_kA=wkA, w_kB=wkB,
        w_v=wv_ext, w_p=np.ascontiguousarray(Wp.T),
        b_qA=bqA, b_qB=bqB, b_kA=bkA, b_kB=bkB,
        b_v=np.ascontiguousarray(np.broadcast_to(bv_ext, (128, 264))),
        b_p=inputs["bp"].astype(f).reshape(C, 1),
        ident=np.eye(128, dtype=f),
    )


def kernel(**inputs) -> np.ndarray:
    inputs = {k: np.asarray(v) for k, v in inputs.items()}
    if "nc" not in _CACHE:
        _CACHE["nc"] = build_nc()
    nc = _CACHE["nc"]

    shared = _prep_host(inputs)
    rgb = np.ascontiguousarray(inputs["rgb"].astype(np.float32).reshape(B, C, N))
    edge = np.ascontiguousarray(inputs["edge"].astype(np.float32).reshape(B, C, N))

    in_maps = []
    for core in range(8):
        b, qs = core // 4, core % 4
        m = dict(shared)
        m["rgb_s"] = np.ascontiguousarray(rgb[b][:, QCH * qs:QCH * (qs + 1)])
        m["edge"] = edge[b]
        in_maps.append(m)

    res = run_bass_kernel_spmd(nc, in_maps, core_ids=list(range(8)))
    full = np.zeros((B, C, N), np.float32)
    for core in range(8):
        b, qs = core // 4, core % 4
        full[b][:, QCH * qs:QCH * (qs + 1)] = res.results[core]["out"]
    return full.reshape(B, C, H, W)
